# revision 15
# baseline (speedup 1.0000x reference)
"""CQAttention Trainium2 kernel.

Math (per batch b, D=128, Lc=1024, Lq=128):
    Ct = C[b].T  (Lc,D);  Qt = Q[b].T  (Lq,D)
    S[c,q] = (Ct[c]*w_m).Qt[q] + Ct[c].w_c + Qt[q].w_q
    S1 = softmax_q(S + qbias), S2 = softmax_c(S + cbias)
    A  = S1 @ Qt
    Bt = (S1 @ S2.T) @ Ct = S1 @ (S2.T @ Ct)        # associativity: W2 := S2.T@Ct is 128x128
    out[b] = concat([Ct, A, Ct*A, Ct*Bt], axis=1).T  # [4D, Lc]

Softmax factorization used on-device (shift-free; logits are O(1)):
    S1[c,q] = exp(s_m + s_q)[c,q] / z1[c],  z1[c] = sum_q exp(s_m+s_q)   (s_c cancels)
    S2[c,q] = exp(s_m + s_c)[c,q] / z2[q],  z2[q] = sum_c exp(s_m+s_c)   (s_q cancels)
Layouts per batch (partition dim first):
    CB  = C[b]                [d, c]   (DMA'd straight in, also output block 0)
    St  = s_m^T               [q, c]   via MM(lhsT=Qw, rhs=CB)
    E1t = exp(St + s_q + qb)  [q, c]   ACT exp with per-partition bias
    S0  = s_m | s_c           [c, q|.] via MM(lhsT=CB_ct, rhs=[Qw|w_c|0])
    E2  = exp(S0 + s_c + cb)  [c, q]   ACT exp with per-partition bias
    W2un|z2 = MM(lhsT=E2_ct, rhs=[CT_ct|1|0]) summed over ct   [q, d|1]
    Z1B = MM(lhsT=ones128, rhs=E1t)    [q->all, c] = z1 broadcast to 128 partitions
    At  = MM(lhsT=QT,  rhs=E1t)        [d, c] (unnormalized A^T)
    Btt = MM(lhsT=W2,  rhs=E1t)        [d, c] (unnormalized Bt^T; W2 pre-scaled by 1/z2)
    blocks: [CB, At*R1, (At*R1)*CB, (Btt*R1)*CB] with R1 = 1/Z1B
"""

import warnings

warnings.filterwarnings("ignore")

import numpy as np

B, D, LC, LQ = 64, 128, 1024, 128
NCORES = 8
NB = B // NCORES  # batches per core
NEG = -1e30

# which matmuls run as float32r (4x faster at N>=256, reduced precision)
CFG = {
    "logits": False,  # st+s0 GEMMs in fp32r (rounds cb -> block0 inexact!)
    "w2": True,       # S2^T @ [Ct|1] in fp32r (rounds e2 + CT tiles)
    "e1": True,       # z1/at/bt GEMMs in fp32r (rounds e1, qt, w2sb, ones)
}

_CACHE = {}


def _build_nc(reps=1):
    import concourse.bass as bass
    import concourse.mybir as mybir
    import concourse.tile as tile
    from concourse import bacc
    from concourse.masks import make_identity

    F32 = mybir.dt.float32
    F32R = mybir.dt.float32r
    AF = mybir.ActivationFunctionType

    nc = bacc.Bacc("TRN2", target_bir_lowering=False, debug=False,
                   num_devices=NCORES)

    Cin = nc.dram_tensor("Cin", [NB, D, LC], F32, kind="ExternalInput")
    Qin = nc.dram_tensor("Qin", [NB, D, LQ], F32, kind="ExternalInput")
    Mb = nc.dram_tensor("Mb", [NB, LQ + LC], F32, kind="ExternalInput")
    W3 = nc.dram_tensor("W3", [3, D], F32, kind="ExternalInput")
    Out = nc.dram_tensor("Out", [NB, 4 * D, LC], F32, kind="ExternalOutput")

    def r(ap, on):
        return ap.bitcast(F32R) if on else ap

    RL = CFG["logits"]   # tags cb, qww
    RW = CFG["w2"]       # tags e2, rhsA
    RE = CFG["e1"]       # tags e1, qt, w2sb, ones128
    NS0 = 256 if RL else 129   # moving width of S0 GEMMs
    NW2 = 256 if RW else 129   # moving width of W2 GEMMs

    NT = LC // D  # 8 c-tiles per batch

    with tile.TileContext(nc) as tc:
        with tc.tile_pool(name="const", bufs=1) as constp, \
             tc.tile_pool(name="pers", bufs=4) as pers, \
             tc.tile_pool(name="io", bufs=2) as iop, \
             tc.tile_pool(name="sb", bufs=2) as sb, \
             tc.tile_pool(name="sm", bufs=3) as sm, \
             tc.tile_pool(name="ps_big", bufs=2, space="PSUM") as ps_big, \
             tc.tile_pool(name="ps_s0", bufs=1, space="PSUM") as ps_s0, \
             tc.tile_pool(name="ps_ct", bufs=1, space="PSUM") as ps_ct, \
             tc.tile_pool(name="ps_qmix", bufs=1, space="PSUM") as ps_qmix:

            # ---- constants ----
            ident = constp.tile([D, D], F32)
            make_identity(nc, ident[:])
            scratch = constp.tile([D, D], F32)   # [1 | zeros...] template
            nc.gpsimd.memset(scratch[:], 0.0)
            nc.gpsimd.memset(scratch[:, 0:1], 1.0)
            ones128 = constp.tile([D, D], F32)
            if RE:
                onesf = constp.tile([D, D], F32)
                nc.gpsimd.memset(onesf[:], 1.0)
                nc.vector.tensor_copy(ones128[:].bitcast(F32R), onesf[:])
            else:
                nc.gpsimd.memset(ones128[:], 1.0)
            wq = constp.tile([D, 1], F32)
            nc.sync.dma_start(wq[:], W3[0, :, None])
            wm = constp.tile([D, 1], F32)
            nc.sync.dma_start(wm[:], W3[2, :, None])

            # persistent double-buffered composite operands:
            # Qww: [Qw | w_c | zeros]   rhsA: per-ct [CT_ct | 1 | zeros]
            qww2 = [pers.tile([D, 256], F32, tag="qww", name=f"qww{i}") for i in range(2)]
            rhs2 = [pers.tile([D, NT, 256], F32, tag="rhsA", name=f"rhsA{i}") for i in range(2)]
            for t in qww2:
                if RL:
                    nc.vector.tensor_copy(t[:, 128:256].bitcast(F32R), scratch[:])
                    # col128 <- w_c (overwrites the template's 1-column)
                    nc.sync.dma_start(t[:, 128:129].bitcast(F32R),
                                      W3[1, :, None].bitcast(F32R))
                else:
                    nc.gpsimd.memset(t[:, 128:256], 0.0)
                    nc.sync.dma_start(t[:, 128:129], W3[1, :, None])
            for t in rhs2:
                # cols 128:256 per ct = [1 | zeros127]
                nc.vector.tensor_copy(
                    r(t[:, :, 128:256], RW),
                    scratch[:, None, :].broadcast_to((D, NT, D)))

            for b in [bb for _ in range(reps) for bb in range(NB)]:
                qww = qww2[b % 2]
                rhsA = rhs2[b % 2]

                # ---- loads ----
                ob = iop.tile([D, 4, LC], F32, tag="ob")  # [CB, A, CtA, CtB]
                cb = ob[:, 0, :]
                nc.sync.dma_start(r(ob[:, 0, 0:512], RL), r(Cin[b, :, 0:512], RL))
                nc.sync.dma_start(r(ob[:, 0, 512:1024], RL),
                                  r(Cin[b, :, 512:1024], RL))
                qb = sb.tile([D, LQ], F32, tag="qb")
                nc.sync.dma_start(qb[:], Qin[b])
                mbt = sm.tile([D, 1 + NT], F32, tag="mbt")
                nc.sync.dma_start(mbt[:], Mb[b].rearrange("(j p) -> p j", p=D))

                # ---- Qw = Q[b] * w_m ----
                nc.vector.tensor_mul(r(qww[:, 0:128], RL), qb[:],
                                     wm[:].to_broadcast((D, LQ)))

                # ---- St = s_m^T [q, c] ----
                p_st = ps_big.tile([D, LC], F32, tag="big")
                nc.tensor.matmul(p_st[:, 0:512], r(qww[:, 0:128], RL),
                                 r(cb[:, 0:512], RL), start=True, stop=True)
                nc.tensor.matmul(p_st[:, 512:1024], r(qww[:, 0:128], RL),
                                 r(cb[:, 512:1024], RL), start=True, stop=True)

                # ---- QT (transpose of Q[b]), s_q column, W2un share one bank ----
                qmix = ps_qmix.tile([D, 4, D], F32, tag="qmix")
                nc.tensor.transpose(qmix[:, 0, :], qb[:], ident[:])
                nc.tensor.matmul(qmix[:, 1, 0:1], qb[:], wq[:], start=True, stop=True)
                sqt = sm.tile([D, 1], F32, tag="sqt")
                nc.vector.tensor_add(sqt[:], qmix[:, 1, 0:1], mbt[:, 0:1])

                # ---- E1t = exp(St + s_q + qbias) ----
                e1 = sb.tile([D, LC], F32, tag="e1")
                nc.scalar.activation(r(e1[:], RE), p_st[:], AF.Exp, bias=sqt[:])

                # ---- CT tiles (transposes of CB) ----
                p_ct = ps_ct.tile([D, NT, D], F32, tag="ct")
                for j in range(NT):
                    nc.tensor.transpose(p_ct[:, j, :], cb[:, j * D:(j + 1) * D],
                                        ident[:])
                nc.scalar.copy(r(rhsA[:, :, 0:128], RW), p_ct[:])
                qt = sb.tile([D, D], F32, tag="qt")
                nc.scalar.copy(r(qt[:], RE), qmix[:, 0, :])

                # ---- S0 tiles: s_m[c,q] | s_c[c]  then E2 = exp(.+bias) ----
                e2 = sb.tile([D, NT, D], F32, tag="e2")
                sc8 = sm.tile([D, NT], F32, tag="sc8")
                for g in range(4):
                    p_s0 = ps_s0.tile([D, 2, 256], F32, tag="s0")
                    for j in range(2):
                        ct = 2 * g + j
                        nc.tensor.matmul(
                            p_s0[:, j, 0:NS0],
                            r(cb[:, ct * D:(ct + 1) * D], RL),
                            r(qww[:, 0:NS0], RL), start=True, stop=True)
                    nc.vector.tensor_add(sc8[:, 2 * g:2 * g + 2],
                                         p_s0[:, :, 128], mbt[:, 1 + 2 * g:3 + 2 * g])
                    for j in range(2):
                        ct = 2 * g + j
                        nc.scalar.activation(r(e2[:, ct, :], RW), p_s0[:, j, 0:128],
                                             AF.Exp, bias=sc8[:, ct:ct + 1])

                # ---- W2un | z2 = sum_ct E2_ct.T @ [CT_ct | 1] ----
                p_w2 = qmix[:, 2:4, :].rearrange("p a b -> p (a b)")
                for ct in range(NT):
                    nc.tensor.matmul(p_w2[:, 0:NW2], r(e2[:, ct, :], RW),
                                     r(rhsA[:, ct, 0:NW2], RW),
                                     start=(ct == 0), stop=(ct == NT - 1))
                r2 = sm.tile([D, 1], F32, tag="r2")
                nc.vector.reciprocal(r2[:], p_w2[:, 128:129])
                w2 = sb.tile([D, D], F32, tag="w2sb")
                nc.vector.tensor_mul(r(w2[:], RE), p_w2[:, 0:128],
                                     r2[:].to_broadcast((D, D)))

                # ---- Z1 broadcast + reciprocal ----
                p_z1 = ps_big.tile([D, LC], F32, tag="big")
                nc.tensor.matmul(p_z1[:, 0:512], r(ones128[:], RE),
                                 r(e1[:, 0:512], RE), start=True, stop=True)
                nc.tensor.matmul(p_z1[:, 512:1024], r(ones128[:], RE),
                                 r(e1[:, 512:1024], RE), start=True, stop=True)
                r1 = sb.tile([D, LC], F32, tag="r1")
                nc.vector.reciprocal_approx_fast(r1[:], p_z1[:])

                # ---- At (unnorm A^T) -> block1, block2 ----
                p_at = ps_big.tile([D, LC], F32, tag="big")
                nc.tensor.matmul(p_at[:, 0:512], r(qt[:], RE),
                                 r(e1[:, 0:512], RE), start=True, stop=True)
                nc.tensor.matmul(p_at[:, 512:1024], r(qt[:], RE),
                                 r(e1[:, 512:1024], RE), start=True, stop=True)
                nc.vector.tensor_mul(ob[:, 1, :], p_at[:], r1[:])
                nc.gpsimd.tensor_mul(ob[:, 2, :], ob[:, 1, :], cb[:])

                # ---- Btt (unnorm Bt^T) -> block3 ----
                p_bt = ps_big.tile([D, LC], F32, tag="big")
                nc.tensor.matmul(p_bt[:, 0:512], r(w2[:], RE),
                                 r(e1[:, 0:512], RE), start=True, stop=True)
                nc.tensor.matmul(p_bt[:, 512:1024], r(w2[:], RE),
                                 r(e1[:, 512:1024], RE), start=True, stop=True)
                t3 = sb.tile([D, LC], F32, tag="t3")
                nc.vector.tensor_mul(t3[:], p_bt[:], r1[:])
                nc.gpsimd.tensor_mul(ob[:, 3, :], t3[:], cb[:])

                # ---- store ----
                for blk in range(4):
                    nc.sync.dma_start(Out[b, blk * D:(blk + 1) * D, :],
                                      ob[:, blk, :])

    nc.compile()
    return nc


def _prep_inmaps(C, Q, cmask, qmask, w):
    C = np.ascontiguousarray(C, dtype=np.float32)
    Q = np.ascontiguousarray(Q, dtype=np.float32)
    mb = np.concatenate(
        [(1.0 - np.asarray(qmask, np.float32)) * NEG,
         (1.0 - np.asarray(cmask, np.float32)) * NEG], axis=1)  # [B, 1152]
    w3 = np.ascontiguousarray(np.asarray(w, np.float32).reshape(3, D))
    in_maps = []
    for k in range(NCORES):
        s = slice(k * NB, (k + 1) * NB)
        in_maps.append({
            "Cin": C[s],
            "Qin": Q[s],
            "Mb": np.ascontiguousarray(mb[s]),
            "W3": w3,
        })
    return in_maps


def _run(C, Q, cmask, qmask, w, trace=False):
    from concourse.bass_utils import run_bass_kernel_spmd

    key = (tuple(sorted(CFG.items())), 1)
    if key not in _CACHE:
        _CACHE[key] = _build_nc()
    nc = _CACHE[key]
    in_maps = _prep_inmaps(C, Q, cmask, qmask, w)
    res = run_bass_kernel_spmd(nc, in_maps, core_ids=list(range(NCORES)),
                               trace=trace)
    out = np.concatenate([res.results[k]["Out"] for k in range(NCORES)], axis=0)
    return out.astype(np.float32, copy=False), res


def kernel(C, Q, cmask, qmask, w):
    out, _ = _run(C, Q, cmask, qmask, w, trace=False)
    return out


# revision 31
# speedup vs baseline: 7.5518x; 7.5518x over previous
"""CQAttention Trainium2 kernel.

Math (per batch b, D=128, Lc=1024, Lq=128):
    Ct = C[b].T  (Lc,D);  Qt = Q[b].T  (Lq,D)
    S[c,q] = (Ct[c]*w_m).Qt[q] + Ct[c].w_c + Qt[q].w_q
    S1 = softmax_q(S + qbias), S2 = softmax_c(S + cbias)
    A  = S1 @ Qt
    Bt = (S1 @ S2.T) @ Ct = S1 @ (S2.T @ Ct)        # associativity: W2 := S2.T@Ct is 128x128
    out[b] = concat([Ct, A, Ct*A, Ct*Bt], axis=1).T  # [4D, Lc]

Softmax factorization used on-device (shift-free; logits are O(1)):
    S1[c,q] = exp(s_m + s_q)[c,q] / z1[c],  z1[c] = sum_q exp(s_m+s_q)   (s_c cancels)
    S2[c,q] = exp(s_m + s_c)[c,q] / z2[q],  z2[q] = sum_c exp(s_m+s_c)   (s_q cancels)
Layouts per batch (partition dim first):
    CB  = C[b]                [d, c]   (DMA'd straight in, also output block 0)
    St  = s_m^T               [q, c]   via MM(lhsT=Qw, rhs=CB)
    E1t = exp(St + s_q + qb)  [q, c]   ACT exp with per-partition bias
    S0  = s_m | s_c           [c, q|.] via MM(lhsT=CB_ct, rhs=[Qw|w_c|0])
    E2  = exp(S0 + s_c + cb)  [c, q]   ACT exp with per-partition bias
    W2un|z2 = MM(lhsT=E2_ct, rhs=[CT_ct|1|0]) summed over ct   [q, d|1]
    Z1B = MM(lhsT=ones128, rhs=E1t)    [q->all, c] = z1 broadcast to 128 partitions
    At  = MM(lhsT=QT,  rhs=E1t)        [d, c] (unnormalized A^T)
    Btt = MM(lhsT=W2,  rhs=E1t)        [d, c] (unnormalized Bt^T; W2 pre-scaled by 1/z2)
    blocks: [CB, At*R1, (At*R1)*CB, (Btt*R1)*CB] with R1 = 1/Z1B
"""

import warnings

warnings.filterwarnings("ignore")

import numpy as np

B, D, LC, LQ = 64, 128, 1024, 128
NCORES = 8
NB = B // NCORES  # batches per core
NEG = -1e30

# which matmuls run as float32r (4x faster at N>=256, reduced precision)
CFG = {
    "logits": False,  # st+s0 GEMMs in fp32r (rounds cb -> block0 inexact!)
    "w2": True,       # S2^T @ [Ct|1] in fp32r (rounds e2 + CT tiles)
    "e1": True,       # z1/at/bt GEMMs in fp32r (rounds e1, qt, w2sb, ones)
}

_CACHE = {}


def _build_nc(reps=1):
    import concourse.bass as bass
    import concourse.mybir as mybir
    import concourse.tile as tile
    from concourse import bacc
    from concourse.masks import make_identity

    F32 = mybir.dt.float32
    F32R = mybir.dt.float32r
    AF = mybir.ActivationFunctionType

    nc = bacc.Bacc("TRN2", target_bir_lowering=False, debug=False,
                   num_devices=NCORES)

    Cin = nc.dram_tensor("Cin", [NB, D, LC], F32, kind="ExternalInput")
    Qin = nc.dram_tensor("Qin", [NB, D, LQ], F32, kind="ExternalInput")
    Mb = nc.dram_tensor("Mb", [NB, LQ + LC], F32, kind="ExternalInput")
    W3 = nc.dram_tensor("W3", [3, D], F32, kind="ExternalInput")
    Out = nc.dram_tensor("Out", [NB, 4 * D, LC], F32, kind="ExternalOutput")

    def r(ap, on):
        return ap.bitcast(F32R) if on else ap

    RL = CFG["logits"]   # tags cb, qww
    RW = CFG["w2"]       # tags e2, rhsA
    RE = CFG["e1"]       # tags e1, qt, w2sb, ones128
    NS0 = 256 if RL else 129   # moving width of S0 GEMMs
    NW2 = 256 if RW else 129   # moving width of W2 GEMMs

    NT = LC // D  # 8 c-tiles per batch

    with tile.TileContext(nc) as tc:
        with tc.tile_pool(name="const", bufs=1) as constp, \
             tc.tile_pool(name="pers", bufs=4) as pers, \
             tc.tile_pool(name="io", bufs=2) as iop, \
             tc.tile_pool(name="sb", bufs=2) as sb, \
             tc.tile_pool(name="sm", bufs=3) as sm, \
             tc.tile_pool(name="ps_big", bufs=2, space="PSUM") as ps_big, \
             tc.tile_pool(name="ps_s0", bufs=1, space="PSUM") as ps_s0, \
             tc.tile_pool(name="ps_ct", bufs=1, space="PSUM") as ps_ct, \
             tc.tile_pool(name="ps_qmix", bufs=1, space="PSUM") as ps_qmix:

            # ---- constants ----
            ident = constp.tile([D, D], F32)
            make_identity(nc, ident[:])
            scratch = constp.tile([D, D], F32)   # [1 | zeros...] template
            nc.gpsimd.memset(scratch[:], 0.0)
            nc.gpsimd.memset(scratch[:, 0:1], 1.0)
            ones128 = constp.tile([D, D], F32)
            if RE:
                onesf = constp.tile([D, D], F32)
                nc.gpsimd.memset(onesf[:], 1.0)
                nc.vector.tensor_copy(ones128[:].bitcast(F32R), onesf[:])
            else:
                nc.gpsimd.memset(ones128[:], 1.0)
            wq = constp.tile([D, 1], F32)
            nc.sync.dma_start(wq[:], W3[0, :, None])
            wm = constp.tile([D, 1], F32)
            nc.sync.dma_start(wm[:], W3[2, :, None])

            # persistent double-buffered composite operands:
            # Qww: [Qw | w_c | zeros]   rhsA: per-ct [CT_ct | 1 | zeros]
            qww2 = [pers.tile([D, 256], F32, tag="qww", name=f"qww{i}") for i in range(2)]
            rhs2 = [pers.tile([D, NT, 256], F32, tag="rhsA", name=f"rhsA{i}") for i in range(2)]
            for t in qww2:
                if RL:
                    nc.vector.tensor_copy(t[:, 128:256].bitcast(F32R), scratch[:])
                    # col128 <- w_c (overwrites the template's 1-column)
                    nc.sync.dma_start(t[:, 128:129].bitcast(F32R),
                                      W3[1, :, None].bitcast(F32R))
                else:
                    nc.gpsimd.memset(t[:, 128:256], 0.0)
                    nc.sync.dma_start(t[:, 128:129], W3[1, :, None])
            for t in rhs2:
                # cols 128:256 per ct = [1 | zeros127]
                nc.vector.tensor_copy(
                    r(t[:, :, 128:256], RW),
                    scratch[:, None, :].broadcast_to((D, NT, D)))

            for b in [bb for _ in range(reps) for bb in range(NB)]:
                qww = qww2[b % 2]
                rhsA = rhs2[b % 2]

                # ---- loads ----
                ob = iop.tile([D, 4, LC], F32, tag="ob")  # [CB, A, CtA, CtB]
                cb = ob[:, 0, :]
                nc.sync.dma_start(r(ob[:, 0, 0:512], RL), r(Cin[b, :, 0:512], RL))
                nc.sync.dma_start(r(ob[:, 0, 512:1024], RL),
                                  r(Cin[b, :, 512:1024], RL))
                qb = sb.tile([D, LQ], F32, tag="qb")
                nc.sync.dma_start(qb[:], Qin[b])
                mbt = sm.tile([D, 1 + NT], F32, tag="mbt")
                nc.sync.dma_start(mbt[:], Mb[b].rearrange("(j p) -> p j", p=D))
                nc.sync.dma_start(Out[b, 0:D, :], ob[:, 0, :])

                # ---- Qw = Q[b] * w_m ----
                nc.vector.tensor_mul(r(qww[:, 0:128], RL), qb[:],
                                     wm[:].to_broadcast((D, LQ)))

                # ---- St = s_m^T [q, c] ----
                p_st = ps_big.tile([D, LC], F32, tag="big")
                for h in range(2):
                    nc.tensor.matmul(p_st[:, 512 * h:512 * (h + 1)],
                                     r(qww[:, 0:128], RL),
                                     r(cb[:, 512 * h:512 * (h + 1)], RL),
                                     start=True, stop=True)

                # ---- QT (transpose of Q[b]), s_q column, W2un share one bank ----
                qmix = ps_qmix.tile([D, 4, D], F32, tag="qmix")
                nc.tensor.transpose(qmix[:, 0, :], qb[:], ident[:])
                nc.tensor.matmul(qmix[:, 1, 0:1], qb[:], wq[:], start=True, stop=True)
                sqt = sm.tile([D, 1], F32, tag="sqt")
                nc.vector.tensor_add(sqt[:], qmix[:, 1, 0:1], mbt[:, 0:1])

                # ---- E1t = exp(St + s_q + qbias) ----
                e1 = sb.tile([D, LC], F32, tag="e1")
                nc.scalar.activation(r(e1[:], RE), p_st[:], AF.Exp, bias=sqt[:])

                # ---- CT tiles (transposes of CB) ----
                p_ct = ps_ct.tile([D, NT, D], F32, tag="ct")
                for j in range(NT):
                    nc.tensor.transpose(p_ct[:, j, :], cb[:, j * D:(j + 1) * D],
                                        ident[:])
                nc.scalar.copy(r(rhsA[:, :, 0:128], RW), p_ct[:])
                qt = sb.tile([D, D], F32, tag="qt")
                nc.scalar.copy(r(qt[:], RE), qmix[:, 0, :])

                # ---- S0 tiles: s_m[c,q] | s_c[c]  then E2 = exp(.+bias) ----
                e2 = sb.tile([D, NT, D], F32, tag="e2")
                sc8 = sm.tile([D, NT], F32, tag="sc8")
                for g in range(4):
                    p_s0 = ps_s0.tile([D, 2, 256], F32, tag="s0")
                    for j in range(2):
                        ct = 2 * g + j
                        nc.tensor.matmul(
                            p_s0[:, j, 0:NS0],
                            r(cb[:, ct * D:(ct + 1) * D], RL),
                            r(qww[:, 0:NS0], RL), start=True, stop=True)
                    nc.vector.tensor_add(sc8[:, 2 * g:2 * g + 2],
                                         p_s0[:, :, 128], mbt[:, 1 + 2 * g:3 + 2 * g])
                    for j in range(2):
                        ct = 2 * g + j
                        nc.scalar.activation(r(e2[:, ct, :], RW), p_s0[:, j, 0:128],
                                             AF.Exp, bias=sc8[:, ct:ct + 1])

                # ---- W2un | z2 = sum_ct E2_ct.T @ [CT_ct | 1] ----
                p_w2 = qmix[:, 2:4, :].rearrange("p a b -> p (a b)")
                for ct in range(NT):
                    nc.tensor.matmul(p_w2[:, 0:NW2], r(e2[:, ct, :], RW),
                                     r(rhsA[:, ct, 0:NW2], RW),
                                     start=(ct == 0), stop=(ct == NT - 1))
                r2 = sm.tile([D, 1], F32, tag="r2")
                nc.vector.reciprocal(r2[:], p_w2[:, 128:129])
                w2 = sb.tile([D, D], F32, tag="w2sb")
                nc.vector.tensor_mul(r(w2[:], RE), p_w2[:, 0:128],
                                     r2[:].to_broadcast((D, D)))

                # ---- Z1 broadcast + reciprocal ----
                p_z1 = ps_big.tile([D, LC], F32, tag="big")
                for h in range(2):
                    nc.tensor.matmul(p_z1[:, 512 * h:512 * (h + 1)],
                                     r(ones128[:], RE),
                                     r(e1[:, 512 * h:512 * (h + 1)], RE),
                                     start=True, stop=True)
                r1 = sb.tile([D, LC], F32, tag="r1")
                nc.vector.reciprocal_approx_fast(r1[:], p_z1[:])

                # ---- At (unnorm A^T) -> block1, block2 ----
                p_at = ps_big.tile([D, LC], F32, tag="big")
                for h in range(2):
                    nc.tensor.matmul(p_at[:, 512 * h:512 * (h + 1)], r(qt[:], RE),
                                     r(e1[:, 512 * h:512 * (h + 1)], RE),
                                     start=True, stop=True)
                nc.vector.tensor_mul(ob[:, 1, :], p_at[:], r1[:])
                nc.sync.dma_start(Out[b, D:2 * D, :], ob[:, 1, :])
                nc.gpsimd.tensor_mul(ob[:, 2, :], ob[:, 1, :], cb[:])
                nc.sync.dma_start(Out[b, 2 * D:3 * D, :], ob[:, 2, :])

                # ---- Btt (unnorm Bt^T) -> block3 ----
                p_bt = ps_big.tile([D, LC], F32, tag="big")
                for h in range(2):
                    nc.tensor.matmul(p_bt[:, 512 * h:512 * (h + 1)], r(w2[:], RE),
                                     r(e1[:, 512 * h:512 * (h + 1)], RE),
                                     start=True, stop=True)
                t3 = sb.tile([D, LC], F32, tag="t3")
                nc.vector.tensor_mul(t3[:], p_bt[:], r1[:])
                nc.gpsimd.tensor_mul(ob[:, 3, :], t3[:], cb[:])
                nc.sync.dma_start(Out[b, 3 * D:4 * D, :], ob[:, 3, :])

    nc.compile()
    return nc


def _prep_inmaps(C, Q, cmask, qmask, w):
    C = np.ascontiguousarray(C, dtype=np.float32)
    Q = np.ascontiguousarray(Q, dtype=np.float32)
    mb = np.concatenate(
        [(1.0 - np.asarray(qmask, np.float32)) * NEG,
         (1.0 - np.asarray(cmask, np.float32)) * NEG], axis=1)  # [B, 1152]
    w3 = np.ascontiguousarray(np.asarray(w, np.float32).reshape(3, D))
    in_maps = []
    for k in range(NCORES):
        s = slice(k * NB, (k + 1) * NB)
        in_maps.append({
            "Cin": C[s],
            "Qin": Q[s],
            "Mb": np.ascontiguousarray(mb[s]),
            "W3": w3,
        })
    return in_maps


def _run(C, Q, cmask, qmask, w, trace=False):
    from concourse.bass_utils import run_bass_kernel_spmd

    key = (tuple(sorted(CFG.items())), 1)
    if key not in _CACHE:
        _CACHE[key] = _build_nc()
    nc = _CACHE[key]
    in_maps = _prep_inmaps(C, Q, cmask, qmask, w)
    res = run_bass_kernel_spmd(nc, in_maps, core_ids=list(range(NCORES)),
                               trace=trace)
    out = np.concatenate([res.results[k]["Out"] for k in range(NCORES)], axis=0)
    return out.astype(np.float32, copy=False), res


def kernel(C, Q, cmask, qmask, w):
    out, _ = _run(C, Q, cmask, qmask, w, trace=False)
    return out
